# revision 34
# baseline (speedup 1.0000x reference)
"""AttentionPooling (segment softmax + weighted segment-sum) on 8 trn2 cores.

Strategy: shard nodes across cores at segment-aligned cuts (batch is sorted).
fp16 data path: host casts x to fp16 and lays it out per core as the exact
SBUF image [128 part, 8 windows, 124 tiles, 258] (x cols 0:256, ones 256:258)
so each (partition, window) slice is one contiguous 64KB DMA descriptor.
Per 128-node tile j: PE transpose (fp16) -> MLP scores -> exp ->
scatter-matmul (A_e^T @ [x|1]) accumulating [128 seg, 258] f32 in PSUM over a
124-tile window; windows chain via a carried partial row for the straddling
segment. Normalization (U/Z) on device. No collectives; host scatters the
per-window rows into the final [4096, 256] output.
"""

import numpy as np

# ---------------------------------------------------------------- constants
N_FULL = 1_000_000
D = 256
H = 128
G = 4096
NCORES = 8
P = 128

WINDOWS = 8
TPW = 124                   # node tiles (128 nodes each) per window
TILES = WINDOWS * TPW       # 992 tiles per core
NC_PAD = TILES * P          # 126976 padded nodes per core
WIN_NODES = TPW * P         # 15872
OUT_ROWS = WINDOWS * P      # 1024 rows per core
LAG = 5                     # scatter pipeline lag (groups of 4 tiles)
EPS = 1e-30


# ---------------------------------------------------------------- host plan
def _plan(batch):
    """batch: sorted int array [N]. Returns per-core planning dicts."""
    batch = np.asarray(batch).astype(np.int64).ravel()
    n = batch.shape[0]
    # all segment-start positions (including 0 and n)
    change = np.flatnonzero(np.diff(batch)) + 1
    bounds = np.concatenate([[0], change, [n]])
    cuts = [0]
    for c in range(1, NCORES):
        tgt = c * n // NCORES
        i = np.searchsorted(bounds, tgt)
        lo = bounds[i - 1] if i > 0 else bounds[0]
        hi = bounds[min(i, len(bounds) - 1)]
        cut = int(lo if (tgt - lo) <= (hi - tgt) else hi)
        cuts.append(cut)
    cuts.append(n)
    for i in range(NCORES):
        assert cuts[i] < cuts[i + 1], f"empty core shard {i}: {cuts}"
        assert cuts[i + 1] - cuts[i] <= NC_PAD, (
            f"core {i} shard {cuts[i + 1] - cuts[i]} > NC_PAD={NC_PAD}"
        )

    plans = []
    for c in range(NCORES):
        lo, hi = cuts[c], cuts[c + 1]
        n_c = hi - lo
        local = batch[lo:hi]
        rel = np.full(NC_PAD, -1.0, np.float32)
        bases = np.full(WINDOWS, -1, np.int64)
        for w in range(WINDOWS):
            a = w * WIN_NODES
            b = min((w + 1) * WIN_NODES, n_c)
            if a >= n_c:
                continue
            base = int(local[a])
            bases[w] = base
            r = local[a:b] - base
            assert r.min() >= 0 and r.max() < P, (
                f"core {c} window {w}: {P} seg rows exceeded (max rel {r.max()})"
            )
            rel[a:b] = r.astype(np.float32)

        last_seg = int(local[-1])
        onehot = np.zeros((P, WINDOWS), np.float32)
        valid = []  # (global_seg_start, nrows) per window
        for w in range(WINDOWS):
            if bases[w] < 0:
                valid.append((0, 0))
                continue
            nxt = bases[w + 1] if (w + 1 < WINDOWS and bases[w + 1] >= 0) else -1
            if nxt >= 0:
                diff = int(nxt - bases[w])
                assert 0 < diff < P, f"core {c} window {w}: carry diff {diff}"
                onehot[diff, w] = 1.0
                hi_seg = nxt
            else:
                hi_seg = last_seg + 1
            nrows = hi_seg - int(bases[w])
            assert 0 < nrows <= P
            valid.append((int(bases[w]), int(nrows)))

        plans.append(dict(lo=lo, hi=hi, n_c=n_c, rel=rel, onehot=onehot, valid=valid))
    return plans


def _make_in_maps(x, W1, b1, W2, b2, plans):
    x = np.asarray(x, dtype=np.float32)
    W1h = np.ascontiguousarray(np.asarray(W1, np.float32).astype(np.float16))
    b1c = np.ascontiguousarray(np.asarray(b1, np.float32)).reshape(H, 1)
    W2h = np.asarray(W2, np.float32).reshape(H, 1).astype(np.float16)
    W2h = np.ascontiguousarray(np.repeat(W2h, 2, axis=1))
    b2c = np.ascontiguousarray(np.asarray(b2, np.float32)).reshape(1, 1)
    in_maps = []
    for pl in plans:
        xpad = np.zeros((NC_PAD, D + 2), np.float16)
        xpad[: pl["n_c"], :D] = x[pl["lo"] : pl["hi"]]
        xpad[:, D:] = 1.0
        # node (w, p, j) = w*WIN_NODES + p*TPW + j -> img[p, w, j*(D+2):...]
        img = xpad.reshape(WINDOWS, P, TPW, D + 2).transpose(1, 0, 2, 3)
        img = np.ascontiguousarray(img).reshape(P, WINDOWS, TPW * (D + 2))
        # rel_sb[p, w*TPW + j] = rel[node]
        rel_arr = np.ascontiguousarray(
            pl["rel"].reshape(WINDOWS, P, TPW).transpose(1, 0, 2).reshape(P, TILES)
        )
        in_maps.append(
            {
                "img": img,
                "relseg": rel_arr,
                "onehot": pl["onehot"],
                "w1": W1h,
                "b1": b1c,
                "w2": W2h,
                "b2": b2c,
            }
        )
    return in_maps


def _assemble(outs, plans, dtype):
    final = np.zeros((G, D), dtype)
    for pl, o in zip(plans, outs):
        for w, (g0, nrows) in enumerate(pl["valid"]):
            if nrows:
                final[g0 : g0 + nrows] = o[w * P : w * P + nrows]
    return final


# ------------------------------------------------------------ numpy emulator
def _emulate(inputs):
    """Pure-numpy emulation of the device program (for logic validation)."""
    x = np.asarray(inputs["x"], np.float32)
    W1 = np.asarray(inputs["W1"], np.float32).astype(np.float16).astype(np.float32)
    b1 = np.asarray(inputs["b1"], np.float32)
    W2 = np.asarray(inputs["W2"], np.float32).astype(np.float16).astype(np.float32)
    b2 = np.asarray(inputs["b2"], np.float32)
    plans = _plan(inputs["batch"])
    outs = []
    cols = np.arange(P, dtype=np.float32)[None, :]
    for pl in plans:
        xp = np.zeros((NC_PAD, D), np.float32)
        xp[: pl["n_c"]] = x[pl["lo"] : pl["hi"]]
        xp = xp.astype(np.float16).astype(np.float32)  # device fp16 quantize
        rel = pl["rel"]
        h = np.tanh(xp @ W1 + b1[None, :])
        s = (h @ W2[:, :1]).ravel() + float(b2.ravel()[0])
        e = np.exp(s)
        out = np.zeros((OUT_ROWS, D), np.float32)
        carry = np.zeros(D + 1, np.float32)
        for w in range(WINDOWS):
            uz = np.zeros((P, D + 1), np.float32)
            a, b = w * WIN_NODES, (w + 1) * WIN_NODES
            A = (cols == rel[a:b, None]).astype(np.float32) * e[a:b, None]
            A16 = A.astype(np.float16).astype(np.float32)
            uz[:, :D] = A16.T @ xp[a:b]
            uz[:, D] = A16.sum(axis=0)
            uz[0] += carry
            carry = pl["onehot"][:, w] @ uz
            out[w * P : (w + 1) * P] = uz[:, :D] / (uz[:, D : D + 1] + EPS)
        outs.append(out)
    return _assemble(outs, plans, np.float32)


# ------------------------------------------------------------- bass program
_NC_CACHE = {}


def _build_nc():
    if "nc" in _NC_CACHE:
        return _NC_CACHE["nc"]
    import concourse.bacc as bacc
    import concourse.mybir as mybir
    import concourse.tile as tile
    from concourse.masks import make_identity

    f32 = mybir.dt.float32
    f16 = mybir.dt.float16
    AF = mybir.ActivationFunctionType
    ALU = mybir.AluOpType

    assert TPW % 4 == 0

    nc = bacc.Bacc(None, target_bir_lowering=False)

    img_d = nc.dram_tensor(
        "img", [P, WINDOWS, TPW * (D + 2)], f16, kind="ExternalInput"
    )
    rel_d = nc.dram_tensor("relseg", [P, TILES], f32, kind="ExternalInput")
    oh_d = nc.dram_tensor("onehot", [P, WINDOWS], f32, kind="ExternalInput")
    w1_d = nc.dram_tensor("w1", [D, H], f16, kind="ExternalInput")
    b1_d = nc.dram_tensor("b1", [H, 1], f32, kind="ExternalInput")
    w2_d = nc.dram_tensor("w2", [H, 2], f16, kind="ExternalInput")
    b2_d = nc.dram_tensor("b2", [1, 1], f32, kind="ExternalInput")
    out_d = nc.dram_tensor("out", [OUT_ROWS, D], f32, kind="ExternalOutput")

    NG = TPW // 4  # groups of 4 tiles per window

    with tile.TileContext(nc) as tc:
        with (
            tc.tile_pool(name="singles", bufs=1) as singles,
            tc.tile_pool(name="sup", bufs=2) as xpool,
            tc.tile_pool(name="xt_sb", bufs=3) as xt_pool,
            tc.tile_pool(name="hb", bufs=3) as hb_pool,
            tc.tile_pool(name="e", bufs=3) as e_pool,
            tc.tile_pool(name="ae", bufs=LAG + 4) as ae_pool,
            tc.tile_pool(name="flush", bufs=2) as flush_pool,
            tc.tile_pool(name="ps_xt", bufs=2, space="PSUM") as ps_xt,
            tc.tile_pool(name="ps_h", bufs=2, space="PSUM") as ps_h,
            tc.tile_pool(name="ps_small", bufs=1, space="PSUM") as ps_small,
            tc.tile_pool(name="ps_uz", bufs=2, space="PSUM") as ps_uz,
        ):
            ident_f = singles.tile([P, P], f32)
            make_identity(nc, ident_f[:])
            ident = singles.tile([P, P], f16)
            nc.vector.tensor_copy(out=ident[:], in_=ident_f[:])
            iota_i = singles.tile([P, P], mybir.dt.int32)
            nc.gpsimd.iota(iota_i[:], pattern=[[1, P]], base=0, channel_multiplier=0)
            iota_f = singles.tile([P, P], f16)
            nc.vector.tensor_copy(out=iota_f[:], in_=iota_i[:])

            w1_sb = singles.tile([P, 2, H], f16)
            w1_r = w1_d[:].rearrange("(c k) m -> c k m", c=2)
            nc.sync.dma_start(out=w1_sb[:, 0, :], in_=w1_r[0])
            nc.sync.dma_start(out=w1_sb[:, 1, :], in_=w1_r[1])
            b1_sb = singles.tile([P, 1], f32)
            nc.sync.dma_start(out=b1_sb[:], in_=b1_d[:])
            w2_sb = singles.tile([P, 2], f16)
            nc.sync.dma_start(out=w2_sb[:], in_=w2_d[:])
            b2_sb = singles.tile([P, 1], f32)
            nc.sync.dma_start(out=b2_sb[:], in_=b2_d[:].to_broadcast([P, 1]))
            oh_sb = singles.tile([P, WINDOWS], f32)
            nc.sync.dma_start(out=oh_sb[:], in_=oh_d[:])
            rel_sb = singles.tile([P, TILES], f32)
            nc.sync.dma_start(out=rel_sb[:], in_=rel_d[:])
            carry_sb = singles.tile([1, D + 1], f32)
            nc.vector.memset(carry_sb[:], 0.0)

            def emit_scatter(item):
                uz, sup_t, ae_t, g, w = item
                for t in range(4):
                    j = g * 4 + t
                    nc.tensor.matmul(
                        out=uz[:],
                        lhsT=ae_t[:, t, :],
                        rhs=sup_t[:, j, :],
                        start=(j == 0),
                        stop=(j == TPW - 1),
                    )
                if g == NG - 1:
                    emit_flush(uz, w)

            def emit_flush(uz_ps, w):
                uz_sb = flush_pool.tile([P, D + 1], f32)
                nc.vector.tensor_copy(out=uz_sb[:], in_=uz_ps[:, 0 : D + 1])
                nc.vector.tensor_add(
                    out=uz_sb[0:1, :], in0=uz_sb[0:1, :], in1=carry_sb[:]
                )
                c_ps = ps_small.tile([1, D + 1], f32)
                nc.tensor.matmul(
                    out=c_ps[:],
                    lhsT=oh_sb[:, w : w + 1],
                    rhs=uz_sb[:],
                    start=True,
                    stop=True,
                )
                nc.vector.tensor_copy(out=carry_sb[:], in_=c_ps[:])
                recip = flush_pool.tile([P, 1], f32)
                nc.vector.tensor_scalar_add(
                    out=recip[:], in0=uz_sb[:, D : D + 1], scalar1=EPS
                )
                nc.vector.reciprocal(out=recip[:], in_=recip[:])
                outw = flush_pool.tile([P, D], f32)
                nc.vector.tensor_scalar_mul(
                    out=outw[:], in0=uz_sb[:, 0:D], scalar1=recip[:]
                )
                nc.sync.dma_start(out=out_d[w * P : (w + 1) * P, :], in_=outw[:])

            def emit_s_exp_ae(item):
                hb, uz_t, sup_t, g, w = item
                s_ps = ps_small.tile([P, 4, 2], f32)
                for t in range(4):
                    nc.tensor.matmul(
                        out=s_ps[:, t, :],
                        lhsT=hb[:, t, :],
                        rhs=w2_sb[:],
                        start=True,
                        stop=True,
                    )
                e_sb = e_pool.tile([P, 4], f32)
                nc.scalar.activation(
                    out=e_sb[:],
                    in_=s_ps[:, :, 0],
                    func=AF.Exp,
                    bias=b2_sb[:],
                    scale=1.0,
                )
                ae_t = ae_pool.tile([P, 4, P], f16)
                for t in range(4):
                    gt = w * TPW + g * 4 + t
                    nc.vector.tensor_scalar(
                        out=ae_t[:, t, :],
                        in0=iota_f[:],
                        scalar1=rel_sb[:, gt : gt + 1],
                        scalar2=e_sb[:, t : t + 1],
                        op0=ALU.is_equal,
                        op1=ALU.mult,
                    )
                sc_queue.append((uz_t, sup_t, ae_t, g, w))

            sc_queue = []
            s_queue = []
            NQ = 4
            JQ = TPW // NQ
            img_r = img_d[:].rearrange("p w (j c) -> p w j c", j=TPW)
            for w in range(WINDOWS):
                sup = xpool.tile([P, TPW, D + 2], f16)
                for q in range(NQ):
                    jq = slice(q * JQ, (q + 1) * JQ)
                    nc.sync.dma_start(out=sup[:, jq, :], in_=img_r[:, w, jq, :])
                uz_ps = ps_uz.tile([P, D + 2], f32)
                for g in range(NG):
                    # ---- PE transpose x tiles: [nodes, D] -> [D, nodes]
                    xt_sb = xt_pool.tile([P, 2, 4, P], f16)
                    for pair in range(2):
                        xt_ps = ps_xt.tile([P, 2, 2, P], f16)
                        for t2 in range(2):
                            j = g * 4 + pair * 2 + t2
                            for k in range(2):
                                nc.tensor.transpose(
                                    out=xt_ps[:, k, t2, :],
                                    in_=sup[:, j, k * P : (k + 1) * P],
                                    identity=ident[:],
                                )
                        p2 = pair * 2
                        if pair == 0:
                            nc.scalar.activation(
                                out=xt_sb[:, :, p2 : p2 + 2, :],
                                in_=xt_ps[:],
                                func=AF.Copy,
                            )
                        else:
                            nc.vector.tensor_copy(
                                out=xt_sb[:, :, p2 : p2 + 2, :], in_=xt_ps[:]
                            )
                    # ---- deferred scatter fills PE while the xt copy lands
                    if len(sc_queue) > LAG:
                        emit_scatter(sc_queue.pop(0))
                    # ---- h = tanh(x @ W1 + b1) for 4 tiles, layout [hid, 4*nodes]
                    h_ps = ps_h.tile([P, 4, P], f32)
                    for k in range(2):
                        nc.tensor.matmul(
                            out=h_ps[:],
                            lhsT=w1_sb[:, k, :],
                            rhs=xt_sb[:, k, :, :],
                            start=(k == 0),
                            stop=(k == 1),
                        )
                    # ---- s/exp/ae for the previous group (its tanh is done;
                    # emitting before this group's tanh keeps exp early in the
                    # Scalar queue)
                    if s_queue:
                        emit_s_exp_ae(s_queue.pop(0))
                    hb = hb_pool.tile([P, 4, P], f16)
                    nc.scalar.activation(
                        out=hb[:], in_=h_ps[:], func=AF.Tanh, bias=b1_sb[:], scale=1.0
                    )
                    s_queue.append((hb, uz_ps, sup, g, w))
            while s_queue:
                emit_s_exp_ae(s_queue.pop(0))
            while sc_queue:
                emit_scatter(sc_queue.pop(0))

    nc.finalize()
    _NC_CACHE["nc"] = nc
    return nc


def _run(inputs, trace=False):
    from concourse.bass_utils import run_bass_kernel_spmd

    x = inputs["x"]
    plans = _plan(inputs["batch"])
    in_maps = _make_in_maps(
        x, inputs["W1"], inputs["b1"], inputs["W2"], inputs["b2"], plans
    )
    nc = _build_nc()
    res = run_bass_kernel_spmd(
        nc, in_maps, core_ids=list(range(NCORES)), trace=trace
    )
    outs = [r["out"] for r in res.results]
    final = _assemble(outs, plans, np.float32)
    return final, res


def kernel(**inputs):
    return _run(inputs, trace=False)[0]
